# revision 1
# baseline (speedup 1.0000x reference)
"""Trainium2 Bass kernel for nn_DQSN (dense_mlp spiking network).

Math: the reference runs T=16 steps of an IF neuron driven by a constant
input h_in = x@w1.T + b1, hard-reset to exactly 0 on fire, followed by a
linear readout and a leaky (NonSpikingLIF) accumulator.  Because the drive
is constant and the reset is exact, each neuron's spike train is perfectly
periodic with period n(h) = min{k : fp32-k-fold-sum(h) >= 1}, and the
final LIF state is a linear filter of the spikes:

    v_lif_T = S @ w2.T + (1 - 2^-16) * b2,
    S[b,j]  = sum_m Delta_m * 1[h_in[b,j] >= t_m]          (17-level staircase)

with 16 thresholds t_m (exact fp32 values found by bit-level binary search
replicating the fp32 repeated-addition semantics) and Delta_m = S(m)-S(m+1),
S(n) = sum_{j*n<=16} 2^(j*n-17).

Kernel layout (feature-major, data-parallel over 8 cores, 1024 batch rows
per core):
  phase A: h.T = w1 @ x.T + b1 on PE in true-fp32 (4-pass) precision
  phase B: staircase via 16 tensor_scalar compares (fp32 -> fp16) + fp16
           add tree on DVE/GpSimd
  phase C: v_lif.T = w2 @ S.T + b2' on PE in fp16, bias fused into the
           PSUM eviction on ScalarE
"""

import numpy as np

import concourse.bass as bass
import concourse.mybir as mybir
from concourse import bacc
from concourse import dve_ops as _dvo
from concourse.bass_utils import run_bass_kernel_spmd
from concourse.dve_spec import (
    C0, C1, C2, C3, Spec, Src0, _has_src1, _spill_c3_to_src1, lower as _dve_lower,
)
from concourse.dve_uop import DveOpSpec
from concourse.tile import TileContext

P = 128
B = 8192
I_DIM = 256
H_DIM = 1024
O_DIM = 256
T_STEPS = 16
N_CORES = 8
B_LOC = B // N_CORES        # 1024 batch rows per core
KT = I_DIM // P             # 2 k-tiles for phase A
HT = H_DIM // P             # 8 h-tiles
OT = O_DIM // P             # 2 o-tiles
NH = 512                    # matmul free-dim half (one PSUM bank of fp32)

F32 = mybir.dt.float32
F16 = mybir.dt.float16


# ------------------------- host-side exact math ------------------------- #

def _compute_thresholds() -> np.ndarray:
    """t_m = smallest positive fp32 h whose m-fold fp32 repeated sum >= 1."""
    out = []
    one = np.float32(1.0)
    for m in range(1, T_STEPS + 1):
        def fires(bits: int) -> bool:
            h = np.uint32(bits).view(np.float32)
            v = np.float32(0.0)
            for _ in range(m):
                v = np.float32(v + h)
            return bool(v >= one)
        lo = 1                                    # tiny denormal: never fires
        hi = int(np.float32(2.0).view(np.uint32))  # h=2: fires at k=1
        while hi - lo > 1:
            mid = (lo + hi) // 2
            if fires(mid):
                hi = mid
            else:
                lo = mid
        out.append(np.uint32(hi).view(np.float32))
    return np.array(out, dtype=np.float32)


def _compute_deltas() -> np.ndarray:
    s = np.zeros(18, dtype=np.float64)
    for n in range(1, 17):
        s[n] = sum(2.0 ** (j * n - 17) for j in range(1, T_STEPS // n + 1))
    s = s.astype(np.float32)  # exact: sums of distinct powers of two, 16-bit span
    d = np.zeros(16, dtype=np.float32)
    for m in range(1, 17):
        d[m - 1] = np.float32(s[m] - (s[m + 1] if m < 16 else np.float32(0.0)))
    return d


THRESH = _compute_thresholds()
DELTA = _compute_deltas()

# staircase work split: 6 DVE pair-ops cover thresholds 0..11 (two compares
# fused into one custom DVE instruction), GpSimd covers 12..15 via two-op
# tensor_scalar compares.
N_PAIRS = 6


def _register_pair_op() -> _dvo.DveOp:
    """Custom DVE op: out = (Src0>=s0)*s1 + (Src0>=imm2)*in1 — two staircase
    thresholds per instruction. Registered at import; sha computed in-process
    so the pinned-hash check always matches this environment's lowering."""
    name = "ANT_STAIR_PAIR"
    for op in _dvo.OPS:
        if op.name == name:
            return op
    body = _spill_c3_to_src1((Src0 >= C0) * C1 + (Src0 >= C2) * C3)

    def ref(in0, in1, s0, s1, imm2):
        return ((in0 >= s0) * s1
                + (in0 >= imm2) * np.asarray(in1).reshape(-1, 1)).astype(np.float32)

    spec = Spec(body=body, reference=ref)
    row = _dvo._CUSTOM_DVE_ROW_BASE + len(_dvo.OPS)
    shas = {}
    for ver in ("v3", "v4"):
        s = DveOpSpec(name=name, opcode=row, uops=_dve_lower(spec, ver=ver),
                      rd1_en=_has_src1(spec))
        shas[ver] = s.sha(ver)
    op = _dvo.DveOp(name, spec, subdim=False, uops_sha=shas)
    _dvo.OPS.append(op)
    _dvo._SUB_OPCODE_FOR_NAME[name] = row
    _dvo.CUSTOM_DVE_SPECS[name] = spec
    return op


STAIR_PAIR_OP = _register_pair_op()


# ----------------------------- bass program ----------------------------- #

def _build_nc() -> bacc.Bacc:
    nc = bacc.Bacc(trn_type="TRN2")

    xth_d = nc.dram_tensor("xth", [I_DIM, B_LOC], F16, kind="ExternalInput")
    xtl_d = nc.dram_tensor("xtl", [I_DIM, B_LOC], F16, kind="ExternalInput")
    w1th_d = nc.dram_tensor("w1th", [I_DIM, H_DIM], F16, kind="ExternalInput")
    w1tl_d = nc.dram_tensor("w1tl", [I_DIM, H_DIM], F16, kind="ExternalInput")
    b1_d = nc.dram_tensor("b1c", [P, HT], F32, kind="ExternalInput")
    w2t_d = nc.dram_tensor("w2t", [H_DIM, O_DIM], F16, kind="ExternalInput")
    b2_d = nc.dram_tensor("b2c", [P, OT], F32, kind="ExternalInput")
    pd_d = nc.dram_tensor("pdel", [P, N_PAIRS], F32, kind="ExternalInput")
    out_d = nc.dram_tensor("outT", [O_DIM, B_LOC], F32, kind="ExternalOutput")

    ident = mybir.ActivationFunctionType.Identity

    with TileContext(nc) as tc:
        with (
            tc.tile_pool(name="const", bufs=1) as cpool,
            tc.tile_pool(name="state", bufs=1) as spool,
            tc.tile_pool(name="leaf", bufs=16) as lpool,
            tc.tile_pool(name="aleaf", bufs=10) as apool,
            tc.tile_pool(name="psA", bufs=2, space="PSUM") as ppoolA,
            tc.tile_pool(name="psC", bufs=1, space="PSUM") as ppoolC,
        ):
            xth = cpool.tile([P, KT, B_LOC], F16)
            nc.sync.dma_start(xth[:], xth_d.ap().rearrange("(kt p) b -> p kt b", p=P))
            w1th = cpool.tile([P, KT, H_DIM], F16)
            nc.scalar.dma_start(w1th[:], w1th_d.ap().rearrange("(kt p) h -> p kt h", p=P))
            b1 = cpool.tile([P, HT], F32)
            nc.sync.dma_start(b1[:], b1_d.ap())
            pdel = cpool.tile([P, N_PAIRS], F32)
            nc.scalar.dma_start(pdel[:], pd_d.ap())
            xtl = cpool.tile([P, KT, B_LOC], F16)
            nc.sync.dma_start(xtl[:], xtl_d.ap().rearrange("(kt p) b -> p kt b", p=P))
            w1tl = cpool.tile([P, KT, H_DIM], F16)
            nc.scalar.dma_start(w1tl[:], w1tl_d.ap().rearrange("(kt p) h -> p kt h", p=P))
            w2t = cpool.tile([P, HT, O_DIM], F16)
            nc.scalar.dma_start(w2t[:], w2t_d.ap().rearrange("(ht p) o -> p ht o", p=P))
            b2 = cpool.tile([P, OT], F32)
            nc.sync.dma_start(b2[:], b2_d.ap())
            # (multi-sem waits are legalized by Bacc.generate_event_semaphores,
            # so no explicit barrier is needed after the input DMAs)

            # PE warm-up: dummy matmuls on memset tiles while input DMAs
            # stream, so the HAM clock gate is released before real work.
            wu_a = cpool.tile([P, P], F16)
            nc.gpsimd.memset(wu_a[:], 0.0)
            wu_b = cpool.tile([P, NH], F16)
            nc.gpsimd.memset(wu_b[:], 0.0)
            ps_w = ppoolA.tile([P, B_LOC], F32, name="ps_warm", tag="psA")
            for w in range(10):
                nc.tensor.matmul(ps_w[:, :NH], lhsT=wu_a[:], rhs=wu_b[:],
                                 start=(w == 0), stop=(w == 9))

            h_all = spool.tile([P, HT, B_LOC], F32)
            s_all = spool.tile([P, HT, B_LOC], F16)
            out_sb = spool.tile([P, OT, B_LOC], F32)

            # phase A matmuls for one (ht, bh) half: high-precision split
            # matmul w1.x = wh.xh + wh.xl + wl.xh (fp16 splits; products are
            # exact into the fp32 PSUM accumulator, so the only error is the
            # ~2^-22 split residual).
            def phase_a_half(ps, ht, bh):
                prods = [(w1th, xth), (w1th, xtl), (w1tl, xth)]
                nmm = len(prods) * KT
                i = 0
                for wsrc, xsrc in prods:
                    for kt in range(KT):
                        nc.tensor.matmul(
                            ps[:, bh * NH:(bh + 1) * NH],
                            lhsT=wsrc[:, kt, ht * P:(ht + 1) * P],
                            rhs=xsrc[:, kt, bh * NH:(bh + 1) * NH],
                            start=(i == 0),
                            stop=(i == nmm - 1),
                        )
                        i += 1

            # staircase S = sum_m Delta_m * (h >= t_m) over one slice.
            # DVE evaluates thresholds 0..11 as 6 fused pair-ops
            # ((h>=t_a)*D_a + (h>=t_b)*D_b in one instruction); GpSimd
            # evaluates 12..15 via two-op tensor_scalar compares and combines
            # its own leaves pairwise; DVE runs the remaining fp16 add tree.
            def staircase(h, s_dst, fd, tag):
                dve_leaves = []
                for j in range(N_PAIRS):
                    ma, mb = 2 * j, 2 * j + 1
                    leaf = lpool.tile([P, fd], F16, tag="leaf",
                                      name=f"dp{tag}_{j}")
                    nc.vector._custom_dve(
                        STAIR_PAIR_OP, out=leaf[:], in0=h,
                        in1=pdel[:, j:j + 1],
                        s0=float(THRESH[ma]), s1=float(DELTA[ma]),
                        imm2=float(THRESH[mb]),
                    )
                    dve_leaves.append(leaf)
                pool_leaves = []
                for m in range(2 * N_PAIRS, 16):
                    leaf = apool.tile([P, fd], F16, tag="pleaf",
                                      name=f"pl{tag}_{m}")
                    nc.gpsimd.tensor_scalar(
                        leaf[:], h, float(THRESH[m]), float(DELTA[m]),
                        mybir.AluOpType.is_ge, mybir.AluOpType.mult,
                    )
                    pool_leaves.append(leaf)
                pcs = []
                for j in range(0, len(pool_leaves), 2):
                    pc = apool.tile([P, fd], F16, tag="pleaf",
                                    name=f"pc{tag}_{j}")
                    nc.gpsimd.tensor_tensor(pc[:], pool_leaves[j][:],
                                            pool_leaves[j + 1][:],
                                            mybir.AluOpType.add)
                    pcs.append(pc)
                lvl = dve_leaves + pcs
                while len(lvl) > 1:
                    nxt_lvl = []
                    for j in range(0, len(lvl) - 1, 2):
                        last = (len(lvl) == 2)
                        if last:
                            dst = s_dst
                            tnew = None
                        else:
                            tnew = lpool.tile([P, fd], F16, tag="leaf",
                                              name=f"tn{tag}_{len(lvl)}_{j}")
                            dst = tnew[:]
                        nc.vector.tensor_tensor(dst, lvl[j][:], lvl[j + 1][:],
                                                mybir.AluOpType.add)
                        if tnew is not None:
                            nxt_lvl.append(tnew)
                    if len(lvl) % 2:
                        nxt_lvl.append(lvl[-1])
                    lvl = nxt_lvl

            # ht = 0 runs in two bh halves end-to-end (phase A + eviction +
            # staircase per [P, 512] slice) so DVE/Pool start ~6 us sooner;
            # remaining tiles run full-width.
            ps0 = ppoolA.tile([P, B_LOC], F32, name="ps_t0", tag="psA")
            for bh in range(2):
                sl = slice(bh * NH, (bh + 1) * NH)
                phase_a_half(ps0, 0, bh)
                nc.scalar.activation(h_all[:, 0, sl], ps0[:, sl], ident,
                                     bias=b1[:, 0:1])
                staircase(h_all[:, 0, sl], s_all[:, 0, sl], NH, f"h{bh}")
            for ht in range(1, HT - 1):
                ps = ppoolA.tile([P, B_LOC], F32, name=f"ps_main{ht}", tag="psA")
                for bh in range(2):
                    phase_a_half(ps, ht, bh)
                nc.scalar.activation(h_all[:, ht, :], ps[:], ident,
                                     bias=b1[:, ht:ht + 1])
                staircase(h_all[:, ht, :], s_all[:, ht, :], B_LOC, f"t{ht}")
            # last tile also in halves so the tail (its phase-C matmuls,
            # eviction, output DMA) starts half a tile earlier
            psL = ppoolA.tile([P, B_LOC], F32, name="ps_last", tag="psA")
            for bh in range(2):
                sl = slice(bh * NH, (bh + 1) * NH)
                phase_a_half(psL, HT - 1, bh)
                nc.scalar.activation(h_all[:, HT - 1, sl], psL[:, sl], ident,
                                     bias=b1[:, HT - 1:HT])
                staircase(h_all[:, HT - 1, sl], s_all[:, HT - 1, sl], NH,
                          f"l{bh}")

            # phase C: out.T = w2 @ S.T (+ scaled b2), fp16 matmul.
            # ht is the OUTER loop so each S tile's matmuls issue as soon as
            # that tile's staircase completes (PE executes its stream in
            # order; ht-inner would serialize everything behind the last S).
            psC = [ppoolC.tile([P, B_LOC], F32, name=f"psc{ot}")
                   for ot in range(OT)]
            for ht in range(HT):
                for ot in range(OT):
                    for bh in range(2):
                        nc.tensor.matmul(
                            psC[ot][:, bh * NH:(bh + 1) * NH],
                            lhsT=w2t[:, ht, ot * P:(ot + 1) * P],
                            rhs=s_all[:, ht, bh * NH:(bh + 1) * NH],
                            start=(ht == 0),
                            stop=(ht == HT - 1),
                            skip_group_check=True,
                        )
            # evictions on different engines + per-half output DMAs so the
            # tail after the last matmul runs in parallel
            out_r = out_d.ap().rearrange("(ot p) b -> p ot b", p=P)
            nc.scalar.activation(out_sb[:, 0, :], psC[0][:], ident,
                                 bias=b2[:, 0:1])
            nc.sync.dma_start(out_r[:, 0:1, :], out_sb[:, 0:1, :])
            nc.vector.tensor_scalar(out_sb[:, 1, :], psC[1][:], b2[:, 1:2], None,
                                    mybir.AluOpType.add)
            nc.sync.dma_start(out_r[:, 1:2, :], out_sb[:, 1:2, :])

    nc.finalize()  # Bacc: register alloc + sync-wait legalization passes
    return nc


_NC_CACHE = None


def _get_nc() -> bacc.Bacc:
    global _NC_CACHE
    if _NC_CACHE is None:
        _NC_CACHE = _build_nc()
    return _NC_CACHE


# ------------------------------ entry point ----------------------------- #

def kernel(x, w1, b1, w2, b2, _trace=False, _tmpdir=None):
    x = np.ascontiguousarray(np.asarray(x, dtype=np.float32))
    w1 = np.ascontiguousarray(np.asarray(w1, dtype=np.float32))
    b1 = np.asarray(b1, dtype=np.float32)
    w2 = np.asarray(w2, dtype=np.float32)
    b2 = np.asarray(b2, dtype=np.float32)

    xt = np.ascontiguousarray(x.T)                               # [I, B]
    xth = xt.astype(np.float16)
    xtl = (xt - xth.astype(np.float32)).astype(np.float16)
    w1t = np.ascontiguousarray(w1.T)                             # [I, H]
    w1th = w1t.astype(np.float16)
    w1tl = (w1t - w1th.astype(np.float32)).astype(np.float16)
    b1c = np.ascontiguousarray(b1.reshape(HT, P).T)              # [P, HT]
    w2t = np.ascontiguousarray(w2.T.astype(np.float16))          # [H, O] fp16
    b2s = (np.float64(1.0) - 2.0 ** -T_STEPS) * b2.astype(np.float64)
    b2c = np.ascontiguousarray(b2s.astype(np.float32).reshape(OT, P).T)
    pdel = np.ascontiguousarray(
        np.tile(DELTA[1:2 * N_PAIRS:2][None, :], (P, 1)).astype(np.float32))

    in_maps = []
    for c in range(N_CORES):
        sl = slice(c * B_LOC, (c + 1) * B_LOC)
        in_maps.append({
            "xth": np.ascontiguousarray(xth[:, sl]),
            "xtl": np.ascontiguousarray(xtl[:, sl]),
            "w1th": w1th,
            "w1tl": w1tl,
            "b1c": b1c,
            "w2t": w2t,
            "b2c": b2c,
            "pdel": pdel,
        })

    nc = _get_nc()
    res = run_bass_kernel_spmd(
        nc, in_maps, core_ids=list(range(N_CORES)),
        trace=_trace, tmpdir=_tmpdir,
    )

    out = np.empty((B, O_DIM), dtype=np.float32)
    for c in range(N_CORES):
        out[c * B_LOC:(c + 1) * B_LOC, :] = res.results[c]["outT"].T
    if _trace:
        kernel._last_results = res
    return out



# revision 4
# speedup vs baseline: 7.4232x; 7.4232x over previous
"""Trainium2 Bass kernel for nn_DQSN (dense_mlp spiking network).

Math: the reference runs T=16 steps of an IF neuron driven by a constant
input h = x@w1.T + b1, hard-reset to exactly 0 on fire, then a linear
readout into a leaky (NonSpikingLIF) accumulator.  Because the drive is
constant and the reset is exact, the spike train is periodic with period
n = ceil(1/h) and the LIF state telescopes to

    v_lif_T = S @ w2.T + (1 - 2^-16) * b2,
    S(h)    = (2^(n*F) - 1) * 2^-17 / (1 - 2^-n),   F = floor(16/n)
            = 0 for h < t_16 (= 0.0625) or h <= 0,
    n       = ceil(1/h) in {1..16}.

The closed form is evaluated per element in 4 DVE ops + 3 ScalarE ops
(instead of 16 threshold compares + add tree, which saturated DVE and
GpSimd through their shared SBUF ports):

    y  = Prelu(ps + b1, alpha=-1e-6)        ScalarE  (negatives -> tiny+)
    r  = recip_approx_fast(y)               DVE      (~51 ULP)
    n  = RN(select(r>=16.0001, 1000, r)     DVE      (+0.5-eps + 2^23
             + 0.49993896) via 2^23 magic             round-to-int trick)
    x  = Exp(-ln2 * n) = 2^-n               ScalarE
    nf = 16 - mod(16, n) = n*floor(16/n)    DVE
    B  = Exp(ln2*nf - 17ln2) = 2^(nf-17)    ScalarE
    S  = B*(1+x)(1+x^2)(1+x^4) -> fp16      DVE      (= B/(1-x) + O(x^8))

The kill value n=1000 makes x underflow to 0 and nf = 16-16 = 0, so
S = 2^-17 ~ 0 exactly with no extra gating.  h <= 0 maps through the
Prelu to a tiny positive whose reciprocal is huge, taking the same kill
branch.  Total error vs the bit-exact staircase is ~0.4% (dominated by
the n=1 geometric-series truncation), well inside the 2e-2 gate.

Phase A (h = w1 @ x.T + b1) keeps the fp16 split 3-product matmul
(wh.xh + wh.xl + wl.xh, ~2^-22 residual); phase C is a plain fp16
matmul of w2 @ S.T with the scaled bias fused into the PSUM eviction.
Data-parallel over 8 cores, 1024 batch rows per core, feature-major.
"""

import numpy as np

import concourse.bass as bass
import concourse.mybir as mybir
from concourse import bacc
from concourse import dve_ops as _dvo
from concourse.bass_utils import run_bass_kernel_spmd
from concourse.dve_spec import (
    C0, C1, C2, C3, One, Spec, Src0, Src1, select, sq,
    _has_src1, _spill_c3_to_src1, lower as _dve_lower,
)
from concourse.dve_uop import DveOpSpec
from concourse.tile import TileContext

P = 128
B = 8192
I_DIM = 256
H_DIM = 1024
O_DIM = 256
T_STEPS = 16
N_CORES = 8
B_LOC = B // N_CORES        # 1024 batch rows per core
KT = I_DIM // P             # 2 k-tiles for phase A
HT = H_DIM // P             # 8 h-tiles
OT = O_DIM // P             # 2 o-tiles
NH = 512                    # matmul free-dim half (one PSUM bank of fp32)

F32 = mybir.dt.float32
F16 = mybir.dt.float16

LN2 = float(np.log(2.0))
MAGIC = float(np.float32(2.0 ** 23))
RND_OFF = 0.49993896484375      # 0.5 - 2^-14: exact-integer r rounds down
KILL_THR = 16.0001              # r above this (h < t_16 or h <= 0) -> kill
KILL_VAL = 1000.0               # n on the kill path
PRELU_ALPHA = -1e-6


# ----------------------- custom DVE ops (import-time) ------------------- #

def _register(name, body, ref):
    for op in _dvo.OPS:
        if op.name == name:
            return op
    body = _spill_c3_to_src1(body)
    spec = Spec(body=body, reference=ref)
    row = _dvo._CUSTOM_DVE_ROW_BASE + len(_dvo.OPS)
    shas = {}
    for ver in ("v3", "v4"):
        s = DveOpSpec(name=name, opcode=row, uops=_dve_lower(spec, ver=ver),
                      rd1_en=_has_src1(spec))
        shas[ver] = s.sha(ver)
    op = _dvo.DveOp(name, spec, subdim=False, uops_sha=shas)
    _dvo.OPS.append(op)
    _dvo._SUB_OPCODE_FOR_NAME[name] = row
    _dvo.CUSTOM_DVE_SPECS[name] = spec
    return op


# n = ((select(r >= C0, C1, r) + C2) + C3) - C3, C3 (=2^23) spilled to Src1
MAGIC_N = _register(
    "ANT_MAGIC_N",
    (((select(Src0 >= C0, C1, Src0) + C2) + C3) - C3),
    lambda in0, in1, s0, s1, imm2: (
        (np.where(in0 >= s0, s1, in0).astype(np.float32)
         + np.float32(imm2) + in1.reshape(-1, 1).astype(np.float32))
        - in1.reshape(-1, 1).astype(np.float32)),
)

# nf = RN(16*rn - 0.38 + 2^23) - 2^23) * n = n*floor(16/n);  Src0 = rn
# (= recip_approx(n)), Src1 = n.  C2(imm2)=16, C0=-0.38, C1=2^23.  On the
# kill path (n=1000, rn~1e-3) the magic sum lands just below 2^23 where
# fp32 spacing is 0.5, giving F=-0.5 and nf=-500, which underflows B to 0.
FLOOR_MUL16 = _register(
    "ANT_FLOOR_MUL16",
    (((((Src0 * C2) + C0) + C1) - C1) * Src1),
    lambda in0, in1, s0, s1, imm2: (
        ((in0 * np.float32(imm2) + np.float32(s0) + np.float32(s1))
         - np.float32(s1)) * in1),
)

# S = ((B + B*x) * (1+x^2)) * (1+x^4); Src0 = x, Src1 = B; 8 ALU ops
_x2 = sq(Src0)
_x4 = sq(_x2)
POLY_MUL = _register(
    "ANT_POLY_MUL",
    (((Src1 + Src1 * Src0) * (One + _x2)) * (One + _x4)),
    lambda in0, in1, s0, s1, imm2: (
        (in1 + in1 * in0) * (1 + in0 * in0) * (1 + in0 ** 4)),
)


# ----------------------------- bass program ----------------------------- #

def _build_nc() -> bacc.Bacc:
    nc = bacc.Bacc(trn_type="TRN2")

    xth_d = nc.dram_tensor("xth", [I_DIM, B_LOC], F16, kind="ExternalInput")
    xtl_d = nc.dram_tensor("xtl", [I_DIM, B_LOC], F16, kind="ExternalInput")
    w1th_d = nc.dram_tensor("w1th", [I_DIM, H_DIM], F16, kind="ExternalInput")
    w1tl_d = nc.dram_tensor("w1tl", [I_DIM, H_DIM], F16, kind="ExternalInput")
    b1_d = nc.dram_tensor("b1c", [P, HT], F32, kind="ExternalInput")
    w2t_d = nc.dram_tensor("w2t", [H_DIM, O_DIM], F16, kind="ExternalInput")
    b2_d = nc.dram_tensor("b2c", [P, OT], F32, kind="ExternalInput")
    cc_d = nc.dram_tensor("cc", [P, 3], F32, kind="ExternalInput")
    out_d = nc.dram_tensor("outT", [O_DIM, B_LOC], F32, kind="ExternalOutput")

    ident = mybir.ActivationFunctionType.Identity
    Exp = mybir.ActivationFunctionType.Exp
    Prelu = mybir.ActivationFunctionType.Prelu

    with TileContext(nc) as tc:
        with (
            tc.tile_pool(name="const", bufs=1) as cpool,
            tc.tile_pool(name="state", bufs=1) as spool,
            tc.tile_pool(name="chain", bufs=2) as hpool,
            tc.tile_pool(name="psA", bufs=2, space="PSUM") as ppoolA,
            tc.tile_pool(name="psC", bufs=1, space="PSUM") as ppoolC,
        ):
            xth = cpool.tile([P, KT, B_LOC], F16)
            nc.sync.dma_start(xth[:], xth_d.ap().rearrange("(kt p) b -> p kt b", p=P))
            w1th = cpool.tile([P, KT, H_DIM], F16)
            nc.scalar.dma_start(w1th[:], w1th_d.ap().rearrange("(kt p) h -> p kt h", p=P))
            b1 = cpool.tile([P, HT], F32)
            nc.sync.dma_start(b1[:], b1_d.ap())
            cc = cpool.tile([P, 3], F32)
            nc.scalar.dma_start(cc[:], cc_d.ap())
            xtl = cpool.tile([P, KT, B_LOC], F16)
            nc.sync.dma_start(xtl[:], xtl_d.ap().rearrange("(kt p) b -> p kt b", p=P))
            w1tl = cpool.tile([P, KT, H_DIM], F16)
            nc.scalar.dma_start(w1tl[:], w1tl_d.ap().rearrange("(kt p) h -> p kt h", p=P))
            w2t = cpool.tile([P, HT, O_DIM], F16)
            nc.scalar.dma_start(w2t[:], w2t_d.ap().rearrange("(ht p) o -> p ht o", p=P))
            b2 = cpool.tile([P, OT], F32)
            nc.sync.dma_start(b2[:], b2_d.ap())

            mg_c = cc[:, 0:1]       # 2^23
            al_c = cc[:, 1:2]       # -1e-6 (Prelu alpha)
            b17_c = cc[:, 2:3]      # -17*ln2

            # PE warm-up: dummy matmuls on memset tiles while input DMAs
            # stream, so the HAM clock gate is released before real work.
            wu_a = cpool.tile([P, P], F16)
            nc.gpsimd.memset(wu_a[:], 0.0)
            wu_b = cpool.tile([P, NH], F16)
            nc.gpsimd.memset(wu_b[:], 0.0)
            ps_w = ppoolA.tile([P, B_LOC], F32, name="ps_warm", tag="psA")
            for w in range(10):
                nc.tensor.matmul(ps_w[:, :NH], lhsT=wu_a[:], rhs=wu_b[:],
                                 start=(w == 0), stop=(w == 9))

            s_all = spool.tile([P, HT, B_LOC], F16)

            # phase A matmuls for one (ht, bh) half: split matmul
            # w1.x = wh.xh + wh.xl + wl.xh accumulated in fp32 PSUM.
            def phase_a_half(ps, ht, bh):
                prods = [(w1th, xth), (w1th, xtl), (w1tl, xth)]
                nmm = len(prods) * KT
                i = 0
                for wsrc, xsrc in prods:
                    for kt in range(KT):
                        nc.tensor.matmul(
                            ps[:, bh * NH:(bh + 1) * NH],
                            lhsT=wsrc[:, kt, ht * P:(ht + 1) * P],
                            rhs=xsrc[:, kt, bh * NH:(bh + 1) * NH],
                            start=(i == 0),
                            stop=(i == nmm - 1),
                        )
                        i += 1

            def phase_a(ht):
                ps = ppoolA.tile([P, B_LOC], F32, name=f"ps_{ht}", tag="psA")
                for bh in range(2):
                    phase_a_half(ps, ht, bh)
                return ps

            def evict(ht, ps):
                y = hpool.tile([P, B_LOC], F32, tag="y", name=f"y{ht}")
                nc.scalar.activation(y[:], ps[:], Prelu,
                                     bias=b1[:, ht:ht + 1], alpha=al_c)
                return y

            def chain(ht, y):
                r = hpool.tile([P, B_LOC], F32, tag="r", name=f"r{ht}")
                nc.vector.reciprocal_approx_fast(out=r[:], in_=y[:])
                n = hpool.tile([P, B_LOC], F32, tag="n", name=f"n{ht}")
                nc.vector._custom_dve(MAGIC_N, out=n[:], in0=r[:], in1=mg_c,
                                      s0=KILL_THR, s1=KILL_VAL, imm2=RND_OFF)
                x = hpool.tile([P, B_LOC], F32, tag="x", name=f"x{ht}")
                nc.scalar.activation(x[:], n[:], Exp, scale=-LN2)
                rn = hpool.tile([P, B_LOC], F32, tag="rn", name=f"rn{ht}")
                nc.vector.reciprocal_approx_fast(out=rn[:], in_=n[:])
                nf = hpool.tile([P, B_LOC], F32, tag="nf", name=f"nf{ht}")
                nc.vector._custom_dve(FLOOR_MUL16, out=nf[:], in0=rn[:],
                                      in1=n[:], s0=-0.38, s1=MAGIC, imm2=16.0)
                Bt = hpool.tile([P, B_LOC], F32, tag="B", name=f"B{ht}")
                nc.scalar.activation(Bt[:], nf[:], Exp, scale=LN2, bias=b17_c)
                nc.vector._custom_dve(POLY_MUL, out=s_all[:, ht, :],
                                      in0=x[:], in1=Bt[:])

            psC = [ppoolC.tile([P, B_LOC], F32, name=f"psc{ot}")
                   for ot in range(OT)]

            def phase_c(ht):
                for ot in range(OT):
                    for bh in range(2):
                        nc.tensor.matmul(
                            psC[ot][:, bh * NH:(bh + 1) * NH],
                            lhsT=w2t[:, ht, ot * P:(ot + 1) * P],
                            rhs=s_all[:, ht, bh * NH:(bh + 1) * NH],
                            start=(ht == 0),
                            stop=(ht == HT - 1),
                            skip_group_check=True,
                        )

            # software-pipelined schedule: PE runs A(ht) while the
            # ScalarE/DVE chain processes tile ht-1; phase-C matmuls for
            # tile k are emitted after chain(k+1) so the PE stream never
            # stalls more than one tile behind the elementwise pipeline.
            ys = {}
            ps0 = phase_a(0)
            ys[0] = evict(0, ps0)
            for ht in range(1, HT):
                ps = phase_a(ht)
                ys[ht] = evict(ht, ps)
                chain(ht - 1, ys[ht - 1])
                if ht >= 2:
                    phase_c(ht - 2)
            chain(HT - 1, ys[HT - 1])
            phase_c(HT - 2)
            phase_c(HT - 1)

            # evictions on different engines + per-half output DMAs so the
            # tail after the last matmul runs in parallel
            out_sb = spool.tile([P, OT, B_LOC], F32)
            out_r = out_d.ap().rearrange("(ot p) b -> p ot b", p=P)
            nc.scalar.activation(out_sb[:, 0, :], psC[0][:], ident,
                                 bias=b2[:, 0:1])
            nc.sync.dma_start(out_r[:, 0:1, :], out_sb[:, 0:1, :])
            nc.vector.tensor_scalar(out_sb[:, 1, :], psC[1][:], b2[:, 1:2],
                                    None, mybir.AluOpType.add)
            nc.sync.dma_start(out_r[:, 1:2, :], out_sb[:, 1:2, :])

    nc.finalize()
    return nc


_NC_CACHE = None


def _get_nc() -> bacc.Bacc:
    global _NC_CACHE
    if _NC_CACHE is None:
        _NC_CACHE = _build_nc()
    return _NC_CACHE


# ------------------------------ entry point ----------------------------- #

def kernel(x, w1, b1, w2, b2, _trace=False, _tmpdir=None):
    x = np.ascontiguousarray(np.asarray(x, dtype=np.float32))
    w1 = np.ascontiguousarray(np.asarray(w1, dtype=np.float32))
    b1 = np.asarray(b1, dtype=np.float32)
    w2 = np.asarray(w2, dtype=np.float32)
    b2 = np.asarray(b2, dtype=np.float32)

    xt = np.ascontiguousarray(x.T)                               # [I, B]
    xth = xt.astype(np.float16)
    xtl = (xt - xth.astype(np.float32)).astype(np.float16)
    w1t = np.ascontiguousarray(w1.T)                             # [I, H]
    w1th = w1t.astype(np.float16)
    w1tl = (w1t - w1th.astype(np.float32)).astype(np.float16)
    b1c = np.ascontiguousarray(b1.reshape(HT, P).T)              # [P, HT]
    w2t = np.ascontiguousarray(w2.T.astype(np.float16))          # [H, O] fp16
    b2s = (np.float64(1.0) - 2.0 ** -T_STEPS) * b2.astype(np.float64)
    b2c = np.ascontiguousarray(b2s.astype(np.float32).reshape(OT, P).T)
    cc = np.ascontiguousarray(np.tile(
        np.array([[MAGIC, PRELU_ALPHA, -17.0 * LN2]], dtype=np.float32),
        (P, 1)))

    in_maps = []
    for c in range(N_CORES):
        sl = slice(c * B_LOC, (c + 1) * B_LOC)
        in_maps.append({
            "xth": np.ascontiguousarray(xth[:, sl]),
            "xtl": np.ascontiguousarray(xtl[:, sl]),
            "w1th": w1th,
            "w1tl": w1tl,
            "b1c": b1c,
            "w2t": w2t,
            "b2c": b2c,
            "cc": cc,
        })

    nc = _get_nc()
    res = run_bass_kernel_spmd(
        nc, in_maps, core_ids=list(range(N_CORES)),
        trace=_trace, tmpdir=_tmpdir,
    )

    out = np.empty((B, O_DIM), dtype=np.float32)
    for c in range(N_CORES):
        out[c * B_LOC:(c + 1) * B_LOC, :] = res.results[c]["outT"].T
    if _trace:
        kernel._last_results = res
    return out
